# revision 29
# baseline (speedup 1.0000x reference)
"""Trainium2 Bass kernel for nn_ExtractPatchesPositionLayer.

Reference semantics: per image b, bilinear-translate the (522,522,1) padded
object by t = -positions[b] (tfa.translate: out(y,x) = img(y+py, x+px),
zero fill outside), then center-crop 5px -> (512,512,1).

Because the shift is constant per image, floor/frac of the offset give an
integer window start (A,B) into the (zero-margin-padded) image plus four
constant bilinear corner weights:

    out[r, j] = c00*W[r, j] + c01*W[r, j+1] + c10*W[r+1, j] + c11*W[r+1, j+1]
    W[r, c] = pp[A+r, B+c]

Layout trick: SBUF partition p holds FOUR consecutive padded-image rows
(A+4p .. A+4p+3, +1 elem) as ONE contiguous DRAM span (4*wpad+1 elements, a
single ~8.4 KB line-rate DMA descriptor per partition).  The shared
horizontal lerp h = (1-wx)*wt + wx*wt[+1] is computed once over the whole
span (ACT mul + DVE fused madd, all CONTIGUOUS free-dim APs -- DVE runs flat
APs at ~2x the rate of strided 3D ones).  The vertical lerp is then
partition-local, h vs h[+wpad], except each partition's LAST row pair, whose
h row 4 == next partition's h row 0: the otherwise-idle PE recovers it with
a shift-matrix matmul (ps[m,:] = h[m+1, 0:512]) that the final DVE madd
reads straight from PSUM.  The very last output row (needs input row A+512,
outside the spans) is patched on host -- O(B*N) work.  Output: 4 consecutive
526-wide y rows per partition = one contiguous ~8.4 KB descriptor each
(junk cols trimmed on host).

DMA routing (hard-won trace facts):
  * inputs: dynamic HWDGE on the SP ring (runtime reg offsets; descriptors
    spread over all 16 SDMA engines by dest SBUF partition).
  * outputs: SWDGE via gpsimd -- HWDGE sends every SBUF->HBM descriptor to
    SDMA engine 0 (1.4 ms serialized); SWDGE's CounterMachine spreads them.
    8+ KB descriptors avoid SWDGE's 8-byte stub-packet flood seen at 2 KB.
Sharding: batch 256 -> 32 images x 8 cores, embarrassingly parallel.
Measured: 1426 us (baseline banded-matmul PE kernel) -> 192 us; DVE ~155 us
and the 16 SDMA engines ~168 us each are the co-binding resources
(~407 GB/s aggregate HBM traffic, past the documented 358 GB/s per-core).
"""

from dataclasses import dataclass

import numpy as np

import concourse.bacc as bacc
import concourse.bass as bass
import concourse.mybir as mybir
import concourse.tile as tile
from concourse.bass_utils import run_bass_kernel_spmd


@dataclass(frozen=True)
class Cfg:
    bpc: int      # images per core
    n: int        # output height/width
    wpad: int     # padded input height/width (with zero margin)
    xlen: int     # flat padded-input length per core (incl. tail pad)

    @property
    def wrow(self):  # output rows per partition
        return self.n // 128

    @property
    def span(self):  # elements DMA'd per partition (WR rows + 1)
        return self.wrow * self.wpad + 1


def build_nc(cfg: Cfg) -> bass.Bass:
    BPC, N, WPAD = cfg.bpc, cfg.n, cfg.wpad
    WR = cfg.wrow
    SPAN = cfg.span
    WIDE = WR * WPAD  # full-width output row block per partition
    XLEN = cfg.xlen
    f32 = mybir.dt.float32
    i32 = mybir.dt.int32
    MUL = mybir.AluOpType.mult
    ADD = mybir.AluOpType.add

    nc = bacc.Bacc("TRN2", target_bir_lowering=False, debug=False)
    x_d = nc.declare_dram_parameter("x", [1, XLEN], f32, isOutput=False)
    offs_d = nc.declare_dram_parameter("offs", [1, BPC], i32, isOutput=False)
    wmat_d = nc.declare_dram_parameter("wmat", [BPC, 128, 4], f32, isOutput=False)
    smat_d = nc.declare_dram_parameter("smat", [128, 127], f32, isOutput=False)
    y_d = nc.declare_dram_parameter("y", [BPC, N, WPAD], f32, isOutput=True)

    with tile.TileContext(nc) as tc:
        with (
            tc.tile_pool(name="const", bufs=1) as constp,
            tc.tile_pool(name="win", bufs=8) as winp,
            tc.tile_pool(name="hp", bufs=5) as hp,
            tc.tile_pool(name="op", bufs=5) as op,
            tc.tile_pool(name="psp", bufs=8, space="PSUM") as psp,
        ):
            # consts ride the ACT HWDGE ring so the SP ring's FIFO head is
            # the first window DMA (shaves the pipeline ramp)
            wmat_sb = constp.tile([128, BPC * 4], f32, tag="wmat")
            nc.scalar.dma_start(
                wmat_sb[:].rearrange("p (i q) -> p i q", q=4),
                wmat_d[:, :, :].transpose([1, 0, 2]),
            )
            offs_sb = constp.tile([1, BPC], i32, tag="offs")
            nc.scalar.dma_start(offs_sb[:], offs_d[:, :])
            smat_sb = constp.tile([128, 127], f32, tag="smat")
            nc.scalar.dma_start(smat_sb[:], smat_d[:, :])

            # one offset-register pool per HWDGE ring; window loads
            # alternate SP/ACT so descriptor generation for consecutive
            # images runs on both rings in parallel (shaves the ramp)
            pools = []
            for eng_t, eng in ((mybir.EngineType.SP, nc.sync),
                               (mybir.EngineType.Activation, nc.scalar)):
                regs = [nc.alloc_register(eng_t, f"dynoff_{eng_t}_{k}")
                        for k in range(min(8, BPC))]
                svs = [nc.snap(r, donate=True, min_val=0, max_val=XLEN - 1)
                       for r in regs]
                pools.append((eng, regs, svs))

            for i in range(BPC):
                eng, regs, svs = pools[i % 2]
                k = (i // 2) % len(regs)
                eng.reg_load(regs[k], offs_sb[0:1, i: i + 1])
                wt = winp.tile([128, SPAN], f32, tag="wt")
                eng.dma_start(
                    wt[:],
                    bass.AP(x_d, svs[k], [[WR * WPAD, 128], [1, SPAN]]),
                )
                # all operands are full-width CONTIGUOUS slices (junk
                # between rows is computed and trimmed on host): DVE runs
                # flat APs at full rate, strided 3D ones at half rate.
                # Shared horizontal lerp h over the whole span, then a
                # partition-local vertical lerp of h against h-shifted-by-
                # one-row: 4 passes total (2 ACT muls + 2 DVE madds).
                kx0 = wmat_sb[:, 4 * i + 0: 4 * i + 1]
                kx1 = wmat_sb[:, 4 * i + 1: 4 * i + 2]
                ky0 = wmat_sb[:, 4 * i + 2: 4 * i + 3]
                ky1 = wmat_sb[:, 4 * i + 3: 4 * i + 4]

                HL = SPAN - 1  # = WIDE: h rows 0..WR-1
                h0 = hp.tile([128, HL], f32, tag="h0")
                ob = op.tile([128, WIDE], f32, tag="ob")
                ps = psp.tile([127, N], f32, tag="ps")

                nc.scalar.mul(h0[:], wt[:, 0:HL], kx0)
                nc.vector.scalar_tensor_tensor(h0[:], wt[:, 1:HL + 1], kx1,
                                               h0[:], MUL, ADD)
                # h row WR (= next partition's h row 0) via idle-PE partition
                # shift: ps[m, j] = h0[m+1, j].  Global row N-1+1 has no next
                # partition -- that one output row is patched on host.
                nc.tensor.matmul(out=ps[:], lhsT=smat_sb[:, :],
                                 rhs=h0[:, 0:N], start=True, stop=True)
                nc.scalar.mul(ob[:], h0[:], ky0)
                nc.vector.scalar_tensor_tensor(
                    ob[:, 0:(WR - 1) * WPAD],
                    h0[:, WPAD:WR * WPAD], ky1,
                    ob[:, 0:(WR - 1) * WPAD], MUL, ADD)
                nc.vector.scalar_tensor_tensor(
                    ob[0:127, (WR - 1) * WPAD:(WR - 1) * WPAD + N],
                    ps[:], ky1[0:127, :],
                    ob[0:127, (WR - 1) * WPAD:(WR - 1) * WPAD + N], MUL, ADD)

                nc.gpsimd.dma_start(
                    bass.AP(y_d, i * (N * WPAD), [[WIDE, 128], [1, WIDE]]),
                    ob[:],
                )
    nc.compile()
    return nc


def host_prep(padded: np.ndarray, positions: np.ndarray, n_cores: int):
    """Shard + build metadata. padded: (B, npad, npad) f32, positions: (B, 2)."""
    B, npad, _ = padded.shape
    n = npad - 10
    bpc = B // n_cores

    px = positions[:, 0].astype(np.float32)
    py = positions[:, 1].astype(np.float32)
    fy = np.floor(py)
    fx = np.floor(px)
    ay = (5 + fy).astype(np.int64)
    ax = (5 + fx).astype(np.int64)
    wy = (py - fy).astype(np.float32)
    wx = (px - fx).astype(np.float32)

    m_lo = int(max(0, -min(ay.min(), ax.min())))
    m_hi = int(max(0, max(ay.max(), ax.max()) + n + 1 - npad))
    wpad = npad + m_lo + m_hi

    pp = np.zeros((B, wpad, wpad), dtype=np.float32)
    pp[:, m_lo:m_lo + npad, m_lo:m_lo + npad] = padded

    A = ay + m_lo
    Bc = ax + m_lo
    base = (np.arange(B, dtype=np.int64) % bpc) * (wpad * wpad)
    off = base + A * wpad + Bc

    wr = n // 128
    span = wr * wpad + 1
    # flat length incl. tail so the last image's strided span stays in bounds
    need = int(off.max()) + 127 * wr * wpad + span
    xlen = max(bpc * wpad * wpad, need)

    cfg = Cfg(bpc=bpc, n=n, wpad=wpad, xlen=xlen)

    smat = np.zeros((128, 127), dtype=np.float32)
    for m in range(127):
        smat[m + 1, m] = 1.0  # ps[m, j] = sum_k smat[k, m] h[k, j] = h[m+1, j]

    # host-side fixup for the last output row (needs input row A+n, which the
    # 4-row spans don't load)
    ar = np.arange(B)[:, None]
    ci = Bc[:, None] + np.arange(n + 1)[None, :]
    r0 = pp[ar, (A + n - 1)[:, None], ci]  # (B, n+1)
    r1 = pp[ar, (A + n)[:, None], ci]
    h0r = (1 - wx)[:, None] * r0[:, :n] + wx[:, None] * r0[:, 1:]
    h1r = (1 - wx)[:, None] * r1[:, :n] + wx[:, None] * r1[:, 1:]
    last_row = ((1 - wy)[:, None] * h0r + wy[:, None] * h1r).astype(np.float32)

    in_maps = []
    for cidx in range(n_cores):
        sl = slice(cidx * bpc, (cidx + 1) * bpc)
        flat = np.zeros((1, xlen), dtype=np.float32)
        flat[0, :bpc * wpad * wpad] = pp[sl].reshape(-1)
        offs = off[sl].astype(np.int32).reshape(1, bpc)
        wmat = np.empty((bpc, 128, 4), dtype=np.float32)
        wmat[:, :, 0] = (1 - wx)[sl][:, None]
        wmat[:, :, 1] = wx[sl][:, None]
        wmat[:, :, 2] = (1 - wy)[sl][:, None]
        wmat[:, :, 3] = wy[sl][:, None]
        in_maps.append({"x": flat, "offs": offs, "wmat": wmat, "smat": smat})
    return cfg, in_maps, last_row


N_CORES = 8
_nc_cache: dict = {}


def kernel(padded_obj: np.ndarray, positions: np.ndarray) -> np.ndarray:
    padded_obj = np.asarray(padded_obj)
    positions = np.asarray(positions)
    B, npad, _, C = padded_obj.shape
    cfg, in_maps, last_row = host_prep(
        padded_obj.reshape(B, npad, npad).astype(np.float32, copy=False),
        positions, N_CORES)

    nc = _nc_cache.get(cfg)
    if nc is None:
        nc = build_nc(cfg)
        _nc_cache[cfg] = nc

    res = run_bass_kernel_spmd(nc, in_maps, core_ids=list(range(N_CORES)))
    out = np.concatenate([r["y"][:, :, :cfg.n] for r in res.results], axis=0)
    out = np.ascontiguousarray(out)
    out[:, cfg.n - 1, :] = last_row
    return out.reshape(B, cfg.n, cfg.n, 1)


# revision 30
# speedup vs baseline: 1.0372x; 1.0372x over previous
"""Trainium2 Bass kernel for nn_ExtractPatchesPositionLayer.

Reference semantics: per image b, bilinear-translate the (522,522,1) padded
object by t = -positions[b] (tfa.translate: out(y,x) = img(y+py, x+px),
zero fill outside), then center-crop 5px -> (512,512,1).

Because the shift is constant per image, floor/frac of the offset give an
integer window start (A,B) into the (zero-margin-padded) image plus four
constant bilinear corner weights:

    out[r, j] = c00*W[r, j] + c01*W[r, j+1] + c10*W[r+1, j] + c11*W[r+1, j+1]
    W[r, c] = pp[A+r, B+c]

Layout trick: SBUF partition p holds FOUR consecutive padded-image rows
(A+4p .. A+4p+3, +1 elem) as ONE contiguous DRAM span (4*wpad+1 elements, a
single ~8.4 KB line-rate DMA descriptor per partition).  The shared
horizontal lerp h = (1-wx)*wt + wx*wt[+1] is computed once over the whole
span (ACT mul + DVE fused madd, all CONTIGUOUS free-dim APs -- DVE runs flat
APs at ~2x the rate of strided 3D ones).  The vertical lerp is then
partition-local, h vs h[+wpad], except each partition's LAST row pair, whose
h row 4 == next partition's h row 0: the otherwise-idle PE recovers it with
a shift-matrix matmul (ps[m,:] = h[m+1, 0:512]) that the final DVE madd
reads straight from PSUM.  The very last output row (needs input row A+512,
outside the spans) is patched on host -- O(B*N) work.  Output: 4 consecutive
526-wide y rows per partition = one contiguous ~8.4 KB descriptor each
(junk cols trimmed on host).

DMA routing (hard-won trace facts):
  * inputs: dynamic HWDGE on the SP ring (runtime reg offsets; descriptors
    spread over all 16 SDMA engines by dest SBUF partition).
  * outputs: SWDGE via gpsimd -- HWDGE sends every SBUF->HBM descriptor to
    SDMA engine 0 (1.4 ms serialized); SWDGE's CounterMachine spreads them.
    8+ KB descriptors avoid SWDGE's 8-byte stub-packet flood seen at 2 KB.
Sharding: batch 256 -> 32 images x 8 cores, embarrassingly parallel.
Measured: 1426 us (baseline banded-matmul PE kernel) -> 192 us; DVE ~155 us
and the 16 SDMA engines ~168 us each are the co-binding resources
(~407 GB/s aggregate HBM traffic, past the documented 358 GB/s per-core).
"""

from dataclasses import dataclass

import numpy as np

import concourse.bacc as bacc
import concourse.bass as bass
import concourse.mybir as mybir
import concourse.tile as tile
from concourse.bass_utils import run_bass_kernel_spmd


@dataclass(frozen=True)
class Cfg:
    bpc: int      # images per core
    n: int        # output height/width
    wpad: int     # padded input height/width (with zero margin)
    xlen: int     # flat padded-input length per core (incl. tail pad)

    @property
    def wrow(self):  # output rows per partition
        return self.n // 128

    @property
    def span(self):  # elements DMA'd per partition (WR rows + 1)
        return self.wrow * self.wpad + 1


def build_nc(cfg: Cfg) -> bass.Bass:
    BPC, N, WPAD = cfg.bpc, cfg.n, cfg.wpad
    WR = cfg.wrow
    SPAN = cfg.span
    WIDE = WR * WPAD  # full-width output row block per partition
    XLEN = cfg.xlen
    f32 = mybir.dt.float32
    i32 = mybir.dt.int32
    MUL = mybir.AluOpType.mult
    ADD = mybir.AluOpType.add

    nc = bacc.Bacc("TRN2", target_bir_lowering=False, debug=False)
    x_d = nc.declare_dram_parameter("x", [1, XLEN], f32, isOutput=False)
    offs_d = nc.declare_dram_parameter("offs", [1, BPC], i32, isOutput=False)
    wmat_d = nc.declare_dram_parameter("wmat", [BPC, 128, 4], f32, isOutput=False)
    smat_d = nc.declare_dram_parameter("smat", [128, 127], f32, isOutput=False)
    y_d = nc.declare_dram_parameter("y", [BPC, N, WPAD], f32, isOutput=True)

    with tile.TileContext(nc) as tc:
        with (
            tc.tile_pool(name="const", bufs=1) as constp,
            tc.tile_pool(name="win", bufs=8) as winp,
            tc.tile_pool(name="hp", bufs=5) as hp,
            tc.tile_pool(name="op", bufs=5) as op,
            tc.tile_pool(name="psp", bufs=8, space="PSUM") as psp,
        ):
            # consts ride the ACT HWDGE ring so the SP ring's FIFO head is
            # the first window DMA (shaves the pipeline ramp)
            wmat_sb = constp.tile([128, BPC * 4], f32, tag="wmat")
            nc.scalar.dma_start(
                wmat_sb[:].rearrange("p (i q) -> p i q", q=4),
                wmat_d[:, :, :].transpose([1, 0, 2]),
            )
            offs_sb = constp.tile([1, BPC], i32, tag="offs")
            nc.scalar.dma_start(offs_sb[:], offs_d[:, :])
            smat_sb = constp.tile([128, 127], f32, tag="smat")
            nc.scalar.dma_start(smat_sb[:], smat_d[:, :])

            regs = [nc.alloc_register(mybir.EngineType.SP, f"dynoff_{k}")
                    for k in range(min(16, BPC))]
            svs = [nc.snap(r, donate=True, min_val=0, max_val=XLEN - 1)
                   for r in regs]
            nreg = len(regs)

            for i in range(BPC):
                k = i % nreg
                nc.sync.reg_load(regs[k], offs_sb[0:1, i: i + 1])
                wt = winp.tile([128, SPAN], f32, tag="wt")
                nc.sync.dma_start(
                    wt[:],
                    bass.AP(x_d, svs[k], [[WR * WPAD, 128], [1, SPAN]]),
                )
                # all operands are full-width CONTIGUOUS slices (junk
                # between rows is computed and trimmed on host): DVE runs
                # flat APs at full rate, strided 3D ones at half rate.
                # Shared horizontal lerp h over the whole span, then a
                # partition-local vertical lerp of h against h-shifted-by-
                # one-row: 4 passes total (2 ACT muls + 2 DVE madds).
                kx0 = wmat_sb[:, 4 * i + 0: 4 * i + 1]
                kx1 = wmat_sb[:, 4 * i + 1: 4 * i + 2]
                ky0 = wmat_sb[:, 4 * i + 2: 4 * i + 3]
                ky1 = wmat_sb[:, 4 * i + 3: 4 * i + 4]

                HL = SPAN - 1  # = WIDE: h rows 0..WR-1
                h0 = hp.tile([128, HL], f32, tag="h0")
                ob = op.tile([128, WIDE], f32, tag="ob")
                ps = psp.tile([127, N], f32, tag="ps")

                nc.scalar.mul(h0[:], wt[:, 0:HL], kx0)
                nc.vector.scalar_tensor_tensor(h0[:], wt[:, 1:HL + 1], kx1,
                                               h0[:], MUL, ADD)
                # h row WR (= next partition's h row 0) via idle-PE partition
                # shift: ps[m, j] = h0[m+1, j].  Global row N-1+1 has no next
                # partition -- that one output row is patched on host.
                nc.tensor.matmul(out=ps[:], lhsT=smat_sb[:, :],
                                 rhs=h0[:, 0:N], start=True, stop=True)
                nc.scalar.mul(ob[:], h0[:], ky0)
                nc.vector.scalar_tensor_tensor(
                    ob[:, 0:(WR - 1) * WPAD],
                    h0[:, WPAD:WR * WPAD], ky1,
                    ob[:, 0:(WR - 1) * WPAD], MUL, ADD)
                nc.vector.scalar_tensor_tensor(
                    ob[0:127, (WR - 1) * WPAD:(WR - 1) * WPAD + N],
                    ps[:], ky1[0:127, :],
                    ob[0:127, (WR - 1) * WPAD:(WR - 1) * WPAD + N], MUL, ADD)

                nc.gpsimd.dma_start(
                    bass.AP(y_d, i * (N * WPAD), [[WIDE, 128], [1, WIDE]]),
                    ob[:],
                )
    nc.compile()
    return nc


def host_prep(padded: np.ndarray, positions: np.ndarray, n_cores: int):
    """Shard + build metadata. padded: (B, npad, npad) f32, positions: (B, 2)."""
    B, npad, _ = padded.shape
    n = npad - 10
    bpc = B // n_cores

    px = positions[:, 0].astype(np.float32)
    py = positions[:, 1].astype(np.float32)
    fy = np.floor(py)
    fx = np.floor(px)
    ay = (5 + fy).astype(np.int64)
    ax = (5 + fx).astype(np.int64)
    wy = (py - fy).astype(np.float32)
    wx = (px - fx).astype(np.float32)

    m_lo = int(max(0, -min(ay.min(), ax.min())))
    m_hi = int(max(0, max(ay.max(), ax.max()) + n + 1 - npad))
    wpad = npad + m_lo + m_hi

    pp = np.zeros((B, wpad, wpad), dtype=np.float32)
    pp[:, m_lo:m_lo + npad, m_lo:m_lo + npad] = padded

    A = ay + m_lo
    Bc = ax + m_lo
    base = (np.arange(B, dtype=np.int64) % bpc) * (wpad * wpad)
    off = base + A * wpad + Bc

    wr = n // 128
    span = wr * wpad + 1
    # flat length incl. tail so the last image's strided span stays in bounds
    need = int(off.max()) + 127 * wr * wpad + span
    xlen = max(bpc * wpad * wpad, need)

    cfg = Cfg(bpc=bpc, n=n, wpad=wpad, xlen=xlen)

    smat = np.zeros((128, 127), dtype=np.float32)
    for m in range(127):
        smat[m + 1, m] = 1.0  # ps[m, j] = sum_k smat[k, m] h[k, j] = h[m+1, j]

    # host-side fixup for the last output row (needs input row A+n, which the
    # 4-row spans don't load)
    ar = np.arange(B)[:, None]
    ci = Bc[:, None] + np.arange(n + 1)[None, :]
    r0 = pp[ar, (A + n - 1)[:, None], ci]  # (B, n+1)
    r1 = pp[ar, (A + n)[:, None], ci]
    h0r = (1 - wx)[:, None] * r0[:, :n] + wx[:, None] * r0[:, 1:]
    h1r = (1 - wx)[:, None] * r1[:, :n] + wx[:, None] * r1[:, 1:]
    last_row = ((1 - wy)[:, None] * h0r + wy[:, None] * h1r).astype(np.float32)

    in_maps = []
    for cidx in range(n_cores):
        sl = slice(cidx * bpc, (cidx + 1) * bpc)
        flat = np.zeros((1, xlen), dtype=np.float32)
        flat[0, :bpc * wpad * wpad] = pp[sl].reshape(-1)
        offs = off[sl].astype(np.int32).reshape(1, bpc)
        wmat = np.empty((bpc, 128, 4), dtype=np.float32)
        wmat[:, :, 0] = (1 - wx)[sl][:, None]
        wmat[:, :, 1] = wx[sl][:, None]
        wmat[:, :, 2] = (1 - wy)[sl][:, None]
        wmat[:, :, 3] = wy[sl][:, None]
        in_maps.append({"x": flat, "offs": offs, "wmat": wmat, "smat": smat})
    return cfg, in_maps, last_row


N_CORES = 8
_nc_cache: dict = {}


def kernel(padded_obj: np.ndarray, positions: np.ndarray) -> np.ndarray:
    padded_obj = np.asarray(padded_obj)
    positions = np.asarray(positions)
    B, npad, _, C = padded_obj.shape
    cfg, in_maps, last_row = host_prep(
        padded_obj.reshape(B, npad, npad).astype(np.float32, copy=False),
        positions, N_CORES)

    nc = _nc_cache.get(cfg)
    if nc is None:
        nc = build_nc(cfg)
        _nc_cache[cfg] = nc

    res = run_bass_kernel_spmd(nc, in_maps, core_ids=list(range(N_CORES)))
    out = np.concatenate([r["y"][:, :, :cfg.n] for r in res.results], axis=0)
    out = np.ascontiguousarray(out)
    out[:, cfg.n - 1, :] = last_row
    return out.reshape(B, cfg.n, cfg.n, 1)
